# revision 1
# baseline (speedup 1.0000x reference)
"""Trainium2 Bass kernel for the AutoregressiveSplineDeep flow.

Computes 4 steps of a MADE-conditioned monotonic linear-rational-spline flow
over N=131072 2-d samples, data-parallel over 8 NeuronCores.

Structure per core (NS = 16384 samples):
  - dim-0 spline params are constants (MADE mask zeroes that path), so the
    z0 chain is 4 applications of one constant spline: evaluated with a
    per-sample bin search + gpsimd indirect-copy gather from constant tables.
  - dim-1 params come from the masked MLP (256-wide, bf16 matmuls on PE in
    channel-major layout); layer-3 is emitted sample-major (lhsT = h2 chunk)
    so spline tables build directly in samples-on-partitions layout.
  - softmax/cumsum of spline widths/heights via ACT exp + one masked
    tensor_tensor_scan; bin search via broadcast compares; all 7 per-sample
    table gathers in one gpsimd indirect_copy; the rational spline itself is
    evaluated on [128, NS/128] fp32 tiles.
"""

import sys

sys.path.insert(0, "/opt/trn_rl_repo")

import numpy as np
import ml_dtypes

INPUT_DIM = 2
COUNT_BINS = 16
BOUND = 5.0
FLOW_LENGTH = 4
HIDDEN = 256
MIN_BIN = 1e-3
MIN_DERIV = 1e-3
MIN_LAMBDA = 0.025
N_FULL = 131072
N_CORES = 8
NS = N_FULL // N_CORES  # 16384 per core

LEFT, RIGHT = -BOUND, BOUND
SCALE10 = RIGHT - LEFT  # 10
CFREE = 1.0 - MIN_BIN * COUNT_BINS  # 0.984
A10C = SCALE10 * CFREE  # 9.84


def _np_const_spline_tables(b3_even):
    """Mirror the reference's dim-0 (constant) spline tables in float64."""
    r = b3_even.astype(np.float64)
    w0, h0, d0, l0 = r[0:16], r[16:32], r[32:47], r[47:63]

    def soft(v):
        e = np.exp(v - v.max())
        return e / e.sum()

    widths = MIN_BIN + CFREE * soft(w0)
    cw = np.concatenate([[0.0], np.cumsum(widths)])
    cw = SCALE10 * cw + LEFT
    cw[0], cw[-1] = LEFT, RIGHT
    widths_f = np.diff(cw)
    heights = MIN_BIN + CFREE * soft(h0)
    ch = np.concatenate([[0.0], np.cumsum(heights)])
    ch = SCALE10 * ch + LEFT
    ch[0], ch[-1] = LEFT, RIGHT
    heights_f = np.diff(ch)
    deriv = np.concatenate([[1.0], MIN_DERIV + np.log1p(np.exp(d0)), [1.0]])
    lam = MIN_LAMBDA + (1.0 - 2.0 * MIN_LAMBDA) * (1.0 / (1.0 + np.exp(-l0)))
    tab = np.concatenate(
        [cw[0:16], widths_f, ch[0:16], heights_f, deriv[0:16], deriv[1:17], lam]
    ).astype(np.float32)  # [112]
    cmp0 = cw[1:16].astype(np.float32)  # [15]
    return tab, cmp0


def _build_program(ns):
    """Build the SPMD bass program for `ns` samples per core."""
    import concourse.bacc as bacc
    import concourse.tile as tile
    import concourse.mybir as mybir

    F32 = mybir.dt.float32
    BF16 = mybir.dt.bfloat16
    U8 = mybir.dt.uint8
    U16 = mybir.dt.uint16
    AF = mybir.ActivationFunctionType
    ALU = mybir.AluOpType

    CC = ns // 128          # columns of the samples-on-partitions tiles
    G = 16                  # sample-chunks per L3 psum tile (2 psum banks)
    BLK = 128 * G           # 1024 samples per spline stage-1 block
    NBLK = ns // BLK
    NF = ns // 512          # 512-sample F-tiles for L1/L2

    nc = bacc.Bacc("TRN2", target_bir_lowering=False, debug=False,
                   num_devices=N_CORES)

    def din(name, shape, dt=F32):
        return nc.dram_tensor(name, list(shape), dt, kind="ExternalInput").ap()

    def dout(name, shape, dt=F32):
        return nc.dram_tensor(name, list(shape), dt, kind="ExternalOutput").ap()

    # per-core data
    xsp = din("XSP", [128, CC, 2])
    x0rowb = din("X0ROWB", [1, ns], BF16)
    # weights / consts (replicated)
    w1cb = din("W1CB", [1, 256], BF16)
    b1t = din("B1T", [128, 2])
    w2tb = din("W2TB", [128, 2, 2, 128], BF16)
    b2t = din("B2T", [128, 2])
    w3tb = din("W3TB", [128, 2, 63], BF16)
    b3tab = din("B3TAB", [128, 63])
    tab0 = din("TAB0", [128, 112])
    cmp0 = din("CMP0", [128, 15])
    iota16 = din("IOTA16", [128, 16])
    b16t = din("B16T", [128, 15])
    m32 = din("M32", [128, G * 32])
    ident = din("IDENT", [128, 128])
    z0out = dout("Z0OUT", [FLOW_LENGTH, 128, CC])
    z1out = dout("Z1OUT", [FLOW_LENGTH, 128, CC])

    with tile.TileContext(nc) as tc:
        _emit(nc, tc, locals())
    nc.compile()
    return nc


def _emit(nc, tc, t):
    import concourse.mybir as mybir

    F32 = mybir.dt.float32
    BF16 = mybir.dt.bfloat16
    U8 = mybir.dt.uint8
    U16 = mybir.dt.uint16
    AF = mybir.ActivationFunctionType
    ALU = mybir.AluOpType

    CC, G, NBLK, NF = t["CC"], t["G"], t["NBLK"], t["NF"]
    ns = t["ns"]
    xsp, x0rowb = t["xsp"], t["x0rowb"]
    w1cb, b1t, w2tb, b2t, w3tb, b3tab = (
        t["w1cb"], t["b1t"], t["w2tb"], t["b2t"], t["w3tb"], t["b3tab"])
    tab0, cmp0, iota16, b16t, m32, ident = (
        t["tab0"], t["cmp0"], t["iota16"], t["b16t"], t["m32"], t["ident"])
    z0out, z1out = t["z0out"], t["z1out"]

    import contextlib
    ctx = contextlib.ExitStack()
    with ctx:
        consts = ctx.enter_context(tc.tile_pool(name="consts", bufs=1))
        zpool = ctx.enter_context(tc.tile_pool(name="z", bufs=1))
        rows = ctx.enter_context(tc.tile_pool(name="rows", bufs=2))
        pha = ctx.enter_context(tc.tile_pool(name="pha", bufs=1))
        s2pool = ctx.enter_context(tc.tile_pool(name="s2", bufs=2))
        mlp = ctx.enter_context(tc.tile_pool(name="mlp", bufs=4))
        mpsum = ctx.enter_context(tc.tile_pool(name="mp", bufs=1, space="PSUM"))
        s2psum = ctx.enter_context(tc.tile_pool(name="s2p", bufs=1, space="PSUM"))
        l3psum = ctx.enter_context(tc.tile_pool(name="l3p", bufs=2, space="PSUM"))
        blkpool = ctx.enter_context(tc.tile_pool(name="blk", bufs=2))

        # ---- load constants into SBUF
        def cload(ap, shape, dt=F32, tag=None):
            tl = consts.tile(shape, dt, tag=tag or ap.name)
            nc.sync.dma_start(tl[:], ap[:])
            return tl

        cW1 = cload(w1cb, [1, 256], BF16)
        cB1 = cload(b1t, [128, 2])
        cW2 = cload(w2tb, [128, 2, 2, 128], BF16)
        cB2 = cload(b2t, [128, 2])
        cW3 = cload(w3tb, [128, 2, 63], BF16)
        cB3 = cload(b3tab, [128, 63])
        cTAB0 = cload(tab0, [128, 112])
        cCMP0 = cload(cmp0, [128, 15])
        cIOTA = cload(iota16, [128, 16])
        cB16 = cload(b16t, [128, 15])
        cM32 = cload(m32, [128, G * 32])
        cIDENT = cload(ident, [128, 128])

        # ---------------- stage-2: rational spline on [128, CC] tiles -------
        def emit_stage2(q, binf, xc, xog, zdst, dim1):
            """q: dict of [128, CC] APs (xk, wd, yk, ht, d0, d1, lr);
            binf/xc/xog: [128, CC] APs; zdst: [128, CC] SBUF AP (output)."""
            P = s2pool

            def tl(tag):
                return P.tile([128, CC], F32, tag="s2_" + tag, name="s2_" + tag)

            def tt(out, a, b, op):
                nc.vector.tensor_tensor(out, a, b, op)

            if dim1:
                # DK = bin==0 ? 1 : MIN_DERIV + softplus(d0); same for DK1/15
                m0 = P.tile([128, CC], U8, tag="s2_m0")
                nc.vector.tensor_scalar(m0[:], binf, 0.5, None, ALU.is_lt)
                m15 = P.tile([128, CC], U8, tag="s2_m15")
                nc.vector.tensor_scalar(m15[:], binf, 14.5, None, ALU.is_gt)
                one = tl("one")
                nc.vector.memset(one[:], 1.0)
                dk = tl("dk")
                e1 = tl("e1")
                nc.scalar.activation(e1[:], q["d0"], AF.Exp)
                nc.vector.tensor_scalar(e1[:], e1[:], 1.0, None, ALU.add)
                nc.scalar.activation(e1[:], e1[:], AF.Ln)
                nc.vector.tensor_scalar(dk[:], e1[:], MIN_DERIV, None, ALU.add)
                nc.vector.copy_predicated(dk[:], m0[:], one[:])
                dk1 = tl("dk1")
                e2 = tl("e2")
                nc.scalar.activation(e2[:], q["d1"], AF.Exp)
                nc.vector.tensor_scalar(e2[:], e2[:], 1.0, None, ALU.add)
                nc.scalar.activation(e2[:], e2[:], AF.Ln)
                nc.vector.tensor_scalar(dk1[:], e2[:], MIN_DERIV, None, ALU.add)
                nc.vector.copy_predicated(dk1[:], m15[:], one[:])
                # lam = m + (1-2m) * sigmoid(lr); sigmoid = exp(-softplus(-x))
                lam = tl("lam")
                sg = tl("sg")
                nc.scalar.activation(sg[:], q["lr"], AF.Exp, scale=-1.0)
                nc.vector.tensor_scalar(sg[:], sg[:], 1.0, None, ALU.add)
                nc.scalar.activation(sg[:], sg[:], AF.Ln)
                nc.scalar.activation(sg[:], sg[:], AF.Exp, scale=-1.0)
                nc.vector.tensor_scalar(
                    lam[:], sg[:], 1.0 - 2.0 * MIN_LAMBDA, MIN_LAMBDA,
                    ALU.mult, ALU.add)
            else:
                dk, dk1 = tl("dk"), tl("dk1")
                nc.vector.tensor_copy(dk[:], q["d0"])
                nc.vector.tensor_copy(dk1[:], q["d1"])
                lam = tl("lam")
                nc.vector.tensor_copy(lam[:], q["lr"])

            # wb = sqrt(dk/dk1) = exp(0.5*(ln dk - ln dk1))
            lna = tl("lna")
            nc.scalar.activation(lna[:], dk[:], AF.Ln)
            lnb = tl("lnb")
            nc.scalar.activation(lnb[:], dk1[:], AF.Ln)
            wb = tl("wb")
            tt(wb[:], lna[:], lnb[:], ALU.subtract)
            nc.scalar.activation(wb[:], wb[:], AF.Exp, scale=0.5)

            om = tl("om")  # 1 - lam
            nc.vector.tensor_scalar(om[:], lam[:], -1.0, 1.0, ALU.mult, ALU.add)
            ih = tl("ih")
            nc.vector.reciprocal_approx_fast(ih[:], q["ht"])
            iw = tl("iw")
            nc.vector.reciprocal_approx_fast(iw[:], q["wd"])

            ta = tl("ta")
            tt(ta[:], om[:], wb[:], ALU.mult)       # om*wb
            tt(ta[:], ta[:], dk1[:], ALU.mult)      # om*wb*dk1
            tb = tl("tb")
            tt(tb[:], lam[:], dk[:], ALU.mult)      # lam*dk
            wc = tl("wc")
            tt(wc[:], ta[:], tb[:], ALU.add)
            tt(wc[:], wc[:], q["wd"], ALU.mult)
            tt(wc[:], wc[:], ih[:], ALU.mult)       # wc

            yb = tl("yb")
            tt(yb[:], q["yk"], q["ht"], ALU.add)
            lw = tl("lw")
            tt(lw[:], lam[:], wb[:], ALU.mult)
            ycn = tl("ycn")
            tt(ycn[:], lw[:], yb[:], ALU.mult)
            t2 = tl("t2")
            tt(t2[:], om[:], q["yk"], ALU.mult)
            tt(ycn[:], ycn[:], t2[:], ALU.add)
            ycd = tl("ycd")
            tt(ycd[:], om[:], lw[:], ALU.add)
            ycdr = tl("ycdr")
            nc.vector.reciprocal_approx_fast(ycdr[:], ycd[:])
            ycd = ycdr
            yc = tl("yc")
            tt(yc[:], ycn[:], ycd[:], ALU.mult)

            th = tl("th")
            tt(th[:], xc, q["xk"], ALU.subtract)
            tt(th[:], th[:], iw[:], ALU.mult)       # theta
            lmt = tl("lmt")
            tt(lmt[:], lam[:], th[:], ALU.subtract)  # lam - theta
            mleft = P.tile([128, CC], U8, tag="s2_ml")
            tt(mleft[:], th[:], lam[:], ALU.is_le)
            wcyc = tl("wcyc")
            tt(wcyc[:], wc[:], yc[:], ALU.mult)
            omt = tl("omt")
            nc.vector.tensor_scalar(omt[:], th[:], -1.0, 1.0, ALU.mult, ALU.add)
            wbyb = tl("wbyb")
            tt(wbyb[:], wb[:], yb[:], ALU.mult)

            numl = tl("numl")
            tt(numl[:], q["yk"], lmt[:], ALU.mult)
            t3 = tl("t3")
            tt(t3[:], wcyc[:], th[:], ALU.mult)
            tt(numl[:], numl[:], t3[:], ALU.add)
            numr = tl("numr")
            tt(numr[:], wcyc[:], omt[:], ALU.mult)
            t4 = tl("t4")
            tt(t4[:], wbyb[:], lmt[:], ALU.mult)
            tt(numr[:], numr[:], t4[:], ALU.subtract)
            num = tl("num")
            nc.vector.select(num[:], mleft[:], numl[:], numr[:])

            denl = tl("denl")
            tt(denl[:], wc[:], th[:], ALU.mult)
            tt(denl[:], denl[:], lmt[:], ALU.add)
            denr = tl("denr")
            tt(denr[:], wc[:], omt[:], ALU.mult)
            t5 = tl("t5")
            tt(t5[:], wb[:], lmt[:], ALU.mult)
            tt(denr[:], denr[:], t5[:], ALU.subtract)
            den = tl("den")
            nc.vector.select(den[:], mleft[:], denl[:], denr[:])
            denr2 = tl("denr2")
            nc.vector.reciprocal_approx_fast(denr2[:], den[:])
            den = denr2
            y = tl("y")
            tt(y[:], num[:], den[:], ALU.mult)
            # identity outside [-B, B]: inside <=> xc == x
            mins = P.tile([128, CC], U8, tag="s2_mi")
            tt(mins[:], xc, xog, ALU.is_equal)
            nc.vector.tensor_copy(zdst, xog)
            nc.vector.copy_predicated(zdst, mins[:], y[:])

        # ---------------- phase A: the z0 chain (constant spline) ----------
        z0t = [zpool.tile([128, CC], F32, tag=f"z0_{s}", name=f"z0_{s}")
               for s in range(FLOW_LENGTH + 1)]
        nc.sync.dma_start(z0t[0][:], xsp[:, :, 0])
        z1sp0 = zpool.tile([128, CC], F32, tag="z1_0")
        nc.sync.dma_start(z1sp0[:], xsp[:, :, 1])

        rowbs = []
        rb0 = rows.tile([1, ns], BF16, tag="rowb")
        nc.sync.dma_start(rb0[:], x0rowb[:])
        rowbs.append(rb0)

        for s in range(FLOW_LENGTH):
            zin = z0t[s]
            xc0 = pha.tile([128, CC], F32, tag="a_xc")
            nc.vector.tensor_scalar(xc0[:], zin[:], LEFT, RIGHT,
                                    ALU.max, ALU.min)
            cmpt = pha.tile([128, CC, 15], F32, tag="a_cmp")
            nc.vector.tensor_tensor(
                cmpt[:],
                xc0[:].unsqueeze(2).broadcast_to((128, CC, 15)),
                cCMP0[:].unsqueeze(1).broadcast_to((128, CC, 15)),
                ALU.is_ge)
            bin0 = pha.tile([128, CC], F32, tag="a_bin")
            nc.vector.tensor_reduce(bin0[:], cmpt[:], mybir.AxisListType.X,
                                    ALU.add)
            sel0 = pha.tile([128, CC, 7], F32, tag="a_sel0")
            tabv = cTAB0[:].rearrange("p (q k) -> p q k", k=16)
            for bb in range(CC // 16):
                sl = slice(16 * bb, 16 * bb + 16)
                oh0 = pha.tile([128, 16, 16], F32, tag="a_oh0", bufs=2)
                nc.vector.tensor_tensor(
                    oh0[:],
                    cIOTA[:].unsqueeze(1).broadcast_to((128, 16, 16)),
                    bin0[:, sl].unsqueeze(2).broadcast_to((128, 16, 16)),
                    ALU.is_equal)
                gm0 = pha.tile([128, 16, 7, 16], F32, tag="a_gm0")
                nc.gpsimd.tensor_tensor(
                    gm0[:],
                    tabv.unsqueeze(1).broadcast_to((128, 16, 7, 16)),
                    oh0[:].unsqueeze(2).broadcast_to((128, 16, 7, 16)),
                    ALU.mult)
                nc.vector.tensor_reduce(sel0[:, sl, :], gm0[:],
                                        mybir.AxisListType.X, ALU.add)
            q = {k: sel0[:, :, i] for i, k in enumerate(
                ["xk", "wd", "yk", "ht", "d0", "d1", "lr"])}
            emit_stage2(q, bin0[:], xc0[:], zin[:], z0t[s + 1][:], dim1=False)
            nc.sync.dma_start(z0out[s], z0t[s + 1][:])
            if s < FLOW_LENGTH - 1:
                ptr = s2psum.tile([CC, 128], F32, tag="a_tp", name="a_tp")
                nc.tensor.transpose(ptr[:], z0t[s + 1][:], cIDENT[:])
                z0tb = pha.tile([CC, 128], BF16, tag="a_z0tb")
                nc.scalar.copy(z0tb[:], ptr[:])
                rb = rows.tile([1, ns], BF16, tag="rowb")
                nc.sync.dma_start(rb[:], z0tb[:])
                rowbs.append(rb)

        # ---------------- phase B: MLP + dim-1 spline per step -------------
        zprev = z1sp0
        for s in range(FLOW_LENGTH):
            rowb = rowbs[s]
            xcs = s2pool.tile([128, CC], F32, tag="b_xc")
            nc.vector.tensor_scalar(xcs[:], zprev[:], LEFT, RIGHT,
                                    ALU.max, ALU.min)
            binacc = s2pool.tile([128, CC], F32, tag="b_bin")
            selacc = s2pool.tile([128, CC, 4], F32, tag="b_selacc")
            dacc0 = s2pool.tile([128, CC], F32, tag="b_dacc0")
            dacc1 = s2pool.tile([128, CC], F32, tag="b_dacc1")
            lacc = s2pool.tile([128, CC], F32, tag="b_lacc")

            h2tiles = {}
            for f in range(NF):
                h1b = mlp.tile([128, 2, 512], BF16, tag="h1b")
                for c in (0, 1):
                    hp1c = mpsum.tile([128, 512], F32, tag="hp1",
                                      name=f"hp1_{c}")
                    nc.tensor.matmul(hp1c[:], cW1[0:1, 128 * c:128 * c + 128],
                                     rowb[0:1, 512 * f:512 * f + 512],
                                     start=True, stop=True)
                    nc.scalar.activation(h1b[:, c, :], hp1c[:], AF.Relu,
                                         bias=cB1[:, c:c + 1])
                hp2 = mpsum.tile([128, 2, 512], F32, tag="hp2")
                for mc in (0, 1):
                    for kc in (0, 1):
                        nc.tensor.matmul(hp2[:, mc, :], cW2[:, kc, mc, :],
                                         h1b[:, kc, :],
                                         start=(kc == 0), stop=(kc == 1))
                h2b = mlp.tile([128, 2, 512], BF16, tag="h2b")
                nc.scalar.activation(h2b[:, 0, :], hp2[:, 0, :], AF.Relu,
                                     bias=cB2[:, 0:1])
                nc.scalar.activation(h2b[:, 1, :], hp2[:, 1, :], AF.Relu,
                                     bias=cB2[:, 1:2])
                h2tiles[f] = h2b

                if f % 4 != 3:
                    continue
                # ---- L3 + spline stage-1 for block b = f // 4 (2048 smp)
                b = f // 4
                pl3 = l3psum.tile([128, 1024], F32, tag="pl3")
                pl3v4 = pl3[:].rearrange("p (a b c) -> p a b c", a=2, c=64)
                pl3v = None
                for g in range(G):
                    hsrc = h2tiles[f - 3 + (g // 4)]
                    off = 128 * (g % 4)
                    for kc in (0, 1):
                        nc.tensor.matmul(pl3v4[:, g // 8, g % 8, 0:63],
                                         hsrc[:, kc, off:off + 128],
                                         cW3[:, kc, :],
                                         start=(kc == 0), stop=(kc == 1))
                # params + bias -> QP[:, :, 64:127]
                qp = blkpool.tile([128, G * 127], F32, tag="qp")
                qpv = qp[:].rearrange("p (g q) -> p g q", q=127)
                for a in (0, 1):
                    pl3a = pl3[:, 512 * a:512 * a + 512].rearrange(
                        "p (b c) -> p b c", c=64)[:, :, 0:63]
                    qpa = qp[:, 1016 * a:1016 * a + 1016].rearrange(
                        "p (b c) -> p b c", c=127)[:, :, 64:127]
                    nc.vector.scalar_tensor_tensor(
                        qpa, pl3a, 1.0,
                        cB3[:].unsqueeze(1).broadcast_to((128, 8, 63)),
                        ALU.mult, ALU.add)
                ew = blkpool.tile([128, G * 32], F32, tag="ew")
                nc.scalar.activation(
                    ew[:].rearrange("p (a b) -> p a b", b=32),
                    qpv[:, :, 64:96], AF.Exp)
                cs = blkpool.tile([128, G * 32], F32, tag="cs")
                nc.vector.tensor_tensor_scan(cs[:], cM32[:], ew[:], 0.0,
                                             ALU.mult, ALU.add)
                csv = cs[:].rearrange("p (g t k) -> p g t k", t=2, k=16)
                inv = blkpool.tile([128, G, 2], F32, tag="inv")
                nc.vector.reciprocal_approx_fast(inv[:], csv[:, :, :, 15])
                awh = blkpool.tile([128, G, 2], F32, tag="awh")
                nc.vector.tensor_scalar(awh[:], inv[:], A10C, None, ALU.mult)

                for half, nm in ((0, "w"), (1, "h")):
                    base = 32 * half  # cw at [0:16]+base? layout: cw,wid at 0..31; ch,hgt at 32..63
                    cwsl = qpv[:, :, base + 1:base + 16]
                    nc.vector.tensor_tensor(
                        cwsl, csv[:, :, half, 0:15],
                        awh[:, :, half].unsqueeze(2).broadcast_to((128, G, 15)),
                        ALU.mult)
                    nc.vector.tensor_tensor(
                        cwsl, cwsl,
                        cB16[:].unsqueeze(1).broadcast_to((128, G, 15)),
                        ALU.add)
                    nc.vector.memset(qpv[:, :, base], LEFT)
                    nc.vector.tensor_tensor(
                        qpv[:, :, base + 16:base + 31],
                        qpv[:, :, base + 1:base + 16],
                        qpv[:, :, base + 0:base + 15], ALU.subtract)
                    nc.vector.tensor_scalar(
                        qpv[:, :, base + 31], qpv[:, :, base + 15],
                        -1.0, RIGHT, ALU.mult, ALU.add)

                xcb = xcs[:, G * b:G * b + G]
                cmpb = blkpool.tile([128, G, 15], F32, tag="cmpb", bufs=3)
                nc.vector.tensor_tensor(
                    cmpb[:],
                    xcb.unsqueeze(2).broadcast_to((128, G, 15)),
                    qpv[:, :, 1:16], ALU.is_ge)
                nc.vector.tensor_reduce(binacc[:, G * b:G * b + G], cmpb[:],
                                        mybir.AxisListType.X, ALU.add)
                binb = binacc[:, G * b:G * b + G]
                oh1 = blkpool.tile([128, G, 16], F32, tag="oh1", bufs=3)
                nc.vector.tensor_tensor(
                    oh1[:],
                    cIOTA[:].unsqueeze(1).broadcast_to((128, G, 16)),
                    binb.unsqueeze(2).broadcast_to((128, G, 16)),
                    ALU.is_equal)
                gm4 = blkpool.tile([128, G, 4, 16], F32, tag="gm4")
                nc.gpsimd.tensor_tensor(
                    gm4[:],
                    qpv[:, :, 0:64].rearrange("p g (t k) -> p g t k", k=16),
                    oh1[:].unsqueeze(2).broadcast_to((128, G, 4, 16)),
                    ALU.mult)
                nc.vector.tensor_reduce(selacc[:, G * b:G * b + G, :], gm4[:],
                                        mybir.AxisListType.X, ALU.add)
                dm = blkpool.tile([128, G, 16], F32, tag="dm", bufs=3)
                nc.gpsimd.tensor_tensor(dm[:, :, 0:15], qpv[:, :, 96:111],
                                        oh1[:, :, 1:16], ALU.mult)
                nc.gpsimd.memset(dm[:, :, 15], 0.0)
                nc.vector.tensor_reduce(dacc0[:, G * b:G * b + G], dm[:],
                                        mybir.AxisListType.X, ALU.add)
                dm2 = blkpool.tile([128, G, 16], F32, tag="dm2", bufs=3)
                nc.gpsimd.tensor_tensor(dm2[:, :, 0:15], qpv[:, :, 96:111],
                                        oh1[:, :, 0:15], ALU.mult)
                nc.gpsimd.memset(dm2[:, :, 15], 0.0)
                nc.vector.tensor_reduce(dacc1[:, G * b:G * b + G], dm2[:],
                                        mybir.AxisListType.X, ALU.add)
                lm = blkpool.tile([128, G, 16], F32, tag="lm", bufs=3)
                nc.gpsimd.tensor_tensor(lm[:], qpv[:, :, 111:127], oh1[:],
                                        ALU.mult)
                nc.vector.tensor_reduce(lacc[:, G * b:G * b + G], lm[:],
                                        mybir.AxisListType.X, ALU.add)

            q = {"xk": selacc[:, :, 0], "wd": selacc[:, :, 1],
                 "yk": selacc[:, :, 2], "ht": selacc[:, :, 3],
                 "d0": dacc0[:], "d1": dacc1[:], "lr": lacc[:]}
            znext = zpool.tile([128, CC], F32, tag=f"z1_{s + 1}")
            emit_stage2(q, binacc[:], xcs[:], zprev[:], znext[:], dim1=True)
            nc.sync.dma_start(z1out[s], znext[:])
            zprev = znext


_NC_CACHE = {}


def _get_program(ns):
    if ns not in _NC_CACHE:
        _NC_CACHE[ns] = _build_program(ns)
    return _NC_CACHE[ns]


def _make_inputs(x, W1, b1, W2, b2, W3, b3, ns):
    """Host-side preprocessing -> per-core input maps."""
    bf = ml_dtypes.bfloat16
    CC = ns // 128
    G = 16
    n_cores = x.shape[0] // ns

    W3o = W3[1::2, :]          # [63, 256] (odd rows; mask m3 keeps them fully)
    b3o = b3[1::2]             # [63]
    tab0_v, cmp0_v = _np_const_spline_tables(b3[0::2])

    w1cb = W1[:, 0].astype(bf)[None, :]                    # [1, 256]
    b1t = b1.reshape(2, 128).T.astype(np.float32)          # [128, 2]
    w2tb = np.empty((128, 2, 2, 128), dtype=bf)
    for kc in range(2):
        for mc in range(2):
            w2tb[:, kc, mc, :] = W2[128 * mc:128 * mc + 128,
                                    128 * kc:128 * kc + 128].T.astype(bf)
    b2t = b2.reshape(2, 128).T.astype(np.float32)
    w3tb = np.empty((128, 2, 63), dtype=bf)
    for kc in range(2):
        w3tb[:, kc, :] = W3o[:, 128 * kc:128 * kc + 128].T.astype(bf)
    b3tab = np.broadcast_to(b3o.astype(np.float32), (128, 63)).copy()
    tab0 = np.broadcast_to(tab0_v, (128, 112)).copy()
    cmp0 = np.broadcast_to(cmp0_v, (128, 15)).copy()
    iota16 = np.broadcast_to(np.arange(16, dtype=np.float32),
                             (128, 16)).copy()
    b16 = np.broadcast_to(
        (0.01 * np.arange(1, 16) - 5.0).astype(np.float32), (128, 15)).copy()
    m32 = np.tile(np.r_[0.0, np.ones(15)].astype(np.float32), G * 2)
    m32 = np.broadcast_to(m32, (128, G * 32)).copy()
    ident = np.eye(128, dtype=np.float32)

    shared = dict(W1CB=w1cb, B1T=b1t, W2TB=w2tb, B2T=b2t, W3TB=w3tb,
                  B3TAB=b3tab, TAB0=tab0, CMP0=cmp0, IOTA16=iota16,
                  B16T=b16, M32=m32, IDENT=ident)

    in_maps = []
    for c in range(n_cores):
        xs = x[c * ns:(c + 1) * ns]                        # [ns, 2]
        xspc = xs.reshape(CC, 128, 2).transpose(1, 0, 2).copy()
        x0rowb = xs[:, 0].astype(bf)[None, :].copy()
        in_maps.append(dict(XSP=xspc.astype(np.float32), X0ROWB=x0rowb,
                            **shared))
    return in_maps


def _run(x, W1, b1, W2, b2, W3, b3, ns, trace=False):
    from concourse.bass_utils import run_bass_kernel_spmd

    n_cores = x.shape[0] // ns
    nc = _get_program(ns)
    in_maps = _make_inputs(x, W1, b1, W2, b2, W3, b3, ns)
    res = run_bass_kernel_spmd(nc, in_maps, list(range(n_cores)), trace=trace)

    n = x.shape[0]
    zs = np.empty((FLOW_LENGTH + 1, n, 2), np.float32)
    zs[0] = x
    for c in range(n_cores):
        r = res.results[c]
        lo = c * ns
        for s in range(FLOW_LENGTH):
            zs[s + 1, lo:lo + ns, 0] = r["Z0OUT"][s].T.reshape(ns)
            zs[s + 1, lo:lo + ns, 1] = r["Z1OUT"][s].T.reshape(ns)
    return zs, res


def kernel(x, W1, b1, W2, b2, W3, b3):
    x = np.ascontiguousarray(np.asarray(x, dtype=np.float32))
    zs, _ = _run(x, np.asarray(W1, np.float32), np.asarray(b1, np.float32),
                 np.asarray(W2, np.float32), np.asarray(b2, np.float32),
                 np.asarray(W3, np.float32), np.asarray(b3, np.float32),
                 NS)
    return zs

